# revision 11
# baseline (speedup 1.0000x reference)
"""Trainium2 Bass kernel for single-head attention with QKV projections.

Problem: q,k,v [4, 2048, 1024] fp32; w_q/w_k/w_v [1024, 1024]; b_* [1024];
additive mask [1, 2048, 2048].
  query = q @ w_q.T + b_q ; key = k @ w_k.T + b_k ; value = v @ w_v.T + b_v
  att = softmax(query @ key.T / sqrt(D) + mask) ; out = att @ value

Sharding: 8 cores = 4 batches x 2 sequence-halves of q rows (1024 rows per
core). Each core computes the full K/V projection for its batch (duplicated
across the pair) and the attention output for its q rows. Uniform SPMD
program; the mask is applied as data (no causality assumption).

Host-side prep (free, not on-device): transposes q/k/v and the weights so
the contraction dim lands on SBUF partitions, and pre-scales the mask by
sqrt(D) so it can be added to the raw QK^T product before the 1/sqrt(D)
scaling fused into the exp activation.
"""

import math

import numpy as np

import concourse.bass as bass
import concourse.mybir as mybir
import concourse.tile as tile
from concourse import bacc
from concourse.bass_utils import run_bass_kernel_spmd
from concourse.masks import make_identity

B, S, D = 4, 2048, 1024
SQ = S // 2          # q rows per core
P = 128              # partitions
NE = D // P          # 8 feature blocks
NQT = SQ // P        # 8 q tiles per core
NKB = S // P         # 16 key blocks
KC = 512             # key chunk for QK^T matmuls
NKC = S // KC        # 4 key chunks
SCALE = 1.0 / math.sqrt(D)

F32 = mybir.dt.float32
# Matmul operand dtype: float32 = full precision (4 cyc/row);
# float32r = single-pass reduced-precision multiply (1 cyc/row at N>=256).
MM_DT = F32


def mm(ap):
    """View an fp32 AP as the matmul operand dtype."""
    if MM_DT is F32:
        return ap
    return ap.bitcast(MM_DT)


def build_bass():
    # Bacc (not raw Bass): its compile() pass legalizes semaphore waits
    # (move_matmul_waits_to_ldweights + generate_event_semaphores) for the
    # TRN2 1-wait-per-instruction constraint.
    nc = bacc.Bacc("TRN2", target_bir_lowering=False, debug=False, num_devices=8)

    qT = nc.dram_tensor("qT", [D, SQ], F32, kind="ExternalInput")
    kT = nc.dram_tensor("kT", [D, S], F32, kind="ExternalInput")
    vT = nc.dram_tensor("vT", [D, S], F32, kind="ExternalInput")
    wqT = nc.dram_tensor("wqT", [D, D], F32, kind="ExternalInput")
    wkT = nc.dram_tensor("wkT", [D, D], F32, kind="ExternalInput")
    wvT = nc.dram_tensor("wvT", [D, D], F32, kind="ExternalInput")
    bq = nc.dram_tensor("bq", [D], F32, kind="ExternalInput")
    bk = nc.dram_tensor("bk", [D], F32, kind="ExternalInput")
    bv = nc.dram_tensor("bv", [D], F32, kind="ExternalInput")
    maskS = nc.dram_tensor("maskS", [SQ, S], F32, kind="ExternalInput")
    out = nc.dram_tensor("out", [SQ, D], F32, kind="ExternalOutput")

    # [d, s] views with the 1024-wide d axis split into 8 partition blocks
    qT3 = qT.rearrange("(o p) s -> p o s", p=P)
    kT3 = kT.rearrange("(o p) s -> p o s", p=P)
    vT3 = vT.rearrange("(o p) s -> p o s", p=P)
    wqT3 = wqT.rearrange("(o p) e -> p o e", p=P)
    wkT3 = wkT.rearrange("(o p) e -> p o e", p=P)
    wvT3 = wvT.rearrange("(o p) e -> p o e", p=P)
    bq2 = bq.rearrange("(o p) -> p o", p=P)
    bk2 = bk.rearrange("(o p) -> p o", p=P)

    with tile.TileContext(nc) as tc:
        with (
            tc.tile_pool(name="const", bufs=1) as const_pool,
            tc.tile_pool(name="qk_res", bufs=1) as qk_res,
        ):
            identity = const_pool.tile([P, P], F32)
            make_identity(nc, identity)

            # Resident across all phases: keyT (64KB/part) + queryT (32KB/part)
            keyT_sb = qk_res.tile([P, NE, S], F32, tag="keyT")
            queryT_sb = qk_res.tile([P, NE, SQ], F32, tag="queryT")

            # ---- Phases 1+2: K and Q projections (transposed outputs)
            ps_proj_cm = tc.tile_pool(name="ps_proj", bufs=2, space="PSUM")
            ps_proj = ps_proj_cm.__enter__()
            with (
                tc.tile_pool(name="weights", bufs=1) as w_pool,
                tc.tile_pool(name="ins", bufs=2) as in_pool,
                tc.tile_pool(name="bias_qk", bufs=1) as bias_qk,
            ):
                bq_sb = bias_qk.tile([P, NE], F32, tag="bq")
                nc.gpsimd.dma_start(out=bq_sb, in_=bq2)
                bk_sb = bias_qk.tile([P, NE], F32, tag="bk")
                nc.gpsimd.dma_start(out=bk_sb, in_=bk2)

                for which in ("k", "q"):
                    w3, x3, b_sb, dst, n_sc = {
                        "k": (wkT3, kT3, bk_sb, keyT_sb, NKC),
                        "q": (wqT3, qT3, bq_sb, queryT_sb, SQ // KC),
                    }[which]
                    w_sb = w_pool.tile([P, NE, D], F32, tag="w")
                    nc.sync.dma_start(out=w_sb, in_=w3)
                    for sc in range(n_sc):
                        xs = in_pool.tile([P, NE, KC], F32, tag="ins")
                        nc.sync.dma_start(
                            out=xs, in_=x3[:, :, sc * KC:(sc + 1) * KC]
                        )
                        for eb in range(NE):
                            ps = ps_proj.tile([P, KC], F32)
                            for db in range(NE):
                                nc.tensor.matmul(
                                    ps,
                                    mm(w_sb[:, db, eb * P:(eb + 1) * P]),
                                    mm(xs[:, db, :]),
                                    start=(db == 0),
                                    stop=(db == NE - 1),
                                )
                            nc.scalar.activation(
                                out=dst[:, eb, sc * KC:(sc + 1) * KC],
                                in_=ps,
                                func=mybir.ActivationFunctionType.Identity,
                                bias=b_sb[:, eb:eb + 1],
                            )

            with tc.tile_pool(name="v_res", bufs=1) as v_res:
                value_sb = v_res.tile([P, NKB, D], F32, tag="value")

                # ---- Phase 3: V projection -> value_sb [k, e] (natural)
                with (
                    tc.tile_pool(name="wv", bufs=1) as wv_pool,
                    tc.tile_pool(name="vins", bufs=2) as vin_pool,
                    tc.tile_pool(name="bias_v", bufs=1) as bias_v,
                ):
                    bv_bcast = bias_v.tile([P, D], F32, tag="bv")
                    nc.gpsimd.dma_start(
                        out=bv_bcast, in_=bv[None, :].to_broadcast([P, D])
                    )
                    for ec in range(2):
                        wv_sb = wv_pool.tile([P, NE, KC], F32, tag="wv")
                        nc.sync.dma_start(
                            out=wv_sb, in_=wvT3[:, :, ec * KC:(ec + 1) * KC]
                        )
                        for kb in range(NKB):
                            vs = vin_pool.tile([P, NE, P], F32, tag="vins")
                            nc.sync.dma_start(
                                out=vs, in_=vT3[:, :, kb * P:(kb + 1) * P]
                            )
                            ps = ps_proj.tile([P, KC], F32)
                            for db in range(NE):
                                nc.tensor.matmul(
                                    ps,
                                    mm(vs[:, db, :]),
                                    mm(wv_sb[:, db, :]),
                                    start=(db == 0),
                                    stop=(db == NE - 1),
                                )
                            nc.vector.tensor_add(
                                out=value_sb[:, kb, ec * KC:(ec + 1) * KC],
                                in0=ps,
                                in1=bv_bcast[:, ec * KC:(ec + 1) * KC],
                            )

                ps_proj_cm.__exit__(None, None, None)

                # ---- Phase 4: attention per q tile
                with (
                    tc.tile_pool(name="z", bufs=1) as z_pool,
                    tc.tile_pool(name="p", bufs=1) as p_pool,
                    tc.tile_pool(name="mask", bufs=2) as mask_pool,
                    tc.tile_pool(name="pt", bufs=3) as pt_pool,
                    tc.tile_pool(name="stats", bufs=4) as stat_pool,
                    tc.tile_pool(name="outs", bufs=2) as out_pool,
                    tc.tile_pool(name="ps_att", bufs=2, space="PSUM") as ps_att,
                    tc.tile_pool(name="ps_tr", bufs=2, space="PSUM") as ps_tr,
                    tc.tile_pool(name="ps_out", bufs=2, space="PSUM") as ps_out,
                ):
                    for j in range(NQT):
                        z_sb = z_pool.tile([P, S], F32, tag="z")
                        for kc in range(NKC):
                            ps_a = ps_att.tile([P, KC], F32)
                            for eb in range(NE):
                                nc.tensor.matmul(
                                    ps_a,
                                    mm(queryT_sb[:, eb, j * P:(j + 1) * P]),
                                    mm(keyT_sb[:, eb, kc * KC:(kc + 1) * KC]),
                                    start=(eb == 0),
                                    stop=(eb == NE - 1),
                                )
                            mask_t = mask_pool.tile([P, KC], F32, tag="mask")
                            nc.sync.dma_start(
                                out=mask_t,
                                in_=maskS[j * P:(j + 1) * P, kc * KC:(kc + 1) * KC],
                            )
                            # z = raw QK^T + mask*sqrt(D); evict PSUM -> SBUF
                            nc.vector.tensor_add(
                                out=z_sb[:, kc * KC:(kc + 1) * KC],
                                in0=ps_a,
                                in1=mask_t,
                            )

                        m_t = stat_pool.tile([P, 1], F32, tag="m")
                        nc.vector.reduce_max(m_t, z_sb, axis=mybir.AxisListType.X)
                        negm = stat_pool.tile([P, 1], F32, tag="negm")
                        nc.scalar.mul(out=negm, in_=m_t, mul=-SCALE)
                        l_t = stat_pool.tile([P, 1], F32, tag="l")
                        p_sb = p_pool.tile([P, S], F32, tag="p")
                        # p = exp(z/sqrt(D) - m/sqrt(D)); l = rowsum(p)
                        nc.scalar.activation(
                            out=p_sb,
                            in_=z_sb,
                            func=mybir.ActivationFunctionType.Exp,
                            bias=negm,
                            scale=SCALE,
                            accum_out=l_t,
                        )
                        recip_l = stat_pool.tile([P, 1], F32, tag="recip")
                        nc.vector.reciprocal(recip_l, l_t)

                        ps_o = ps_out.tile([P, D], F32)
                        for kb in range(NKB):
                            ps_t = ps_tr.tile([P, P], F32)
                            nc.tensor.transpose(
                                ps_t, p_sb[:, kb * P:(kb + 1) * P], identity
                            )
                            pT_sb = pt_pool.tile([P, P], F32, tag="pt")
                            nc.vector.tensor_copy(pT_sb, ps_t)
                            for ec in range(2):
                                nc.tensor.matmul(
                                    ps_o[:, ec * KC:(ec + 1) * KC],
                                    mm(pT_sb),
                                    mm(value_sb[:, kb, ec * KC:(ec + 1) * KC]),
                                    start=(kb == 0),
                                    stop=(kb == NKB - 1),
                                )
                        out_sb = out_pool.tile([P, D], F32, tag="out")
                        nc.vector.tensor_scalar_mul(
                            out=out_sb, in0=ps_o, scalar1=recip_l
                        )
                        nc.sync.dma_start(out=out[j * P:(j + 1) * P, :], in_=out_sb)

    nc.finalize()
    return nc


_NC_CACHE = None
LAST_RESULT = None  # BassKernelResults from the most recent kernel() call


def kernel(q, k, v, mask, w_q, b_q, w_k, b_k, w_v, b_v):
    global _NC_CACHE, LAST_RESULT
    if _NC_CACHE is None:
        _NC_CACHE = build_bass()
    nc = _NC_CACHE

    f32 = np.float32
    wqT = np.ascontiguousarray(np.asarray(w_q, dtype=f32).T)
    wkT = np.ascontiguousarray(np.asarray(w_k, dtype=f32).T)
    wvT = np.ascontiguousarray(np.asarray(w_v, dtype=f32).T)
    bq = np.ascontiguousarray(np.asarray(b_q, dtype=f32))
    bk = np.ascontiguousarray(np.asarray(b_k, dtype=f32))
    bv = np.ascontiguousarray(np.asarray(b_v, dtype=f32))
    mask = np.asarray(mask, dtype=f32)
    # pre-scale so the kernel can add it to raw QK^T before the fused 1/sqrt(D)
    maskS_halves = [
        np.ascontiguousarray(mask[0, h * SQ:(h + 1) * SQ, :] * f32(math.sqrt(D)))
        for h in range(2)
    ]

    in_maps = []
    for c in range(8):
        b, h = c // 2, c % 2
        rows = slice(h * SQ, (h + 1) * SQ)
        in_maps.append({
            "qT": np.ascontiguousarray(np.asarray(q[b], dtype=f32)[rows, :].T),
            "kT": np.ascontiguousarray(np.asarray(k[b], dtype=f32).T),
            "vT": np.ascontiguousarray(np.asarray(v[b], dtype=f32).T),
            "wqT": wqT, "wkT": wkT, "wvT": wvT,
            "bq": bq, "bk": bk, "bv": bv,
            "maskS": maskS_halves[h],
        })

    res = run_bass_kernel_spmd(nc, in_maps, list(range(8)))
    LAST_RESULT = res

    out = np.empty((B, S, D), dtype=f32)
    for c in range(8):
        b, h = c // 2, c % 2
        out[b, h * SQ:(h + 1) * SQ, :] = res.results[c]["out"]
    return out


# revision 15
# speedup vs baseline: 2.5189x; 2.5189x over previous
"""Trainium2 Bass kernel for single-head attention with QKV projections.

Problem: q,k,v [4, 2048, 1024] fp32; w_q/w_k/w_v [1024, 1024]; b_* [1024];
additive mask [1, 2048, 2048].
  query = q @ w_q.T + b_q ; key = k @ w_k.T + b_k ; value = v @ w_v.T + b_v
  att = softmax(query @ key.T / sqrt(D) + mask) ; out = att @ value

Sharding: 8 cores = 4 batches x 2 sequence-halves of q rows (1024 rows per
core). Each core computes the full K/V projection for its batch (duplicated
across the pair) and the attention output for its q rows. Uniform SPMD
program; the mask is applied as data (no causality assumption).

Host-side prep (free, not on-device): transposes q/k/v and the weights so
the contraction dim lands on SBUF partitions, and pre-scales the mask by
sqrt(D) so it can be added to the raw QK^T product before the 1/sqrt(D)
scaling fused into the exp activation.
"""

import math

import numpy as np

import concourse.bass as bass
import concourse.mybir as mybir
import concourse.tile as tile
from concourse import bacc
from concourse.bass_utils import run_bass_kernel_spmd
from concourse.masks import make_identity

B, S, D = 4, 2048, 1024
SQ = S // 2          # q rows per core
P = 128              # partitions
NE = D // P          # 8 feature blocks
NQT = SQ // P        # 8 q tiles per core
NKB = S // P         # 16 key blocks
KC = 512             # key chunk for QK^T matmuls
NKC = S // KC        # 4 key chunks
SCALE = 1.0 / math.sqrt(D)

F32 = mybir.dt.float32
# Matmul operand dtype: float32 = full precision (4 cyc/row);
# float32r = single-pass reduced-precision multiply (1 cyc/row at N>=256).
MM_DT = mybir.dt.float32r


def mm(ap):
    """Matmul operands are typed MM_DT end-to-end; no view change needed."""
    return ap


def build_bass():
    # Bacc (not raw Bass): its compile() pass legalizes semaphore waits
    # (move_matmul_waits_to_ldweights + generate_event_semaphores) for the
    # TRN2 1-wait-per-instruction constraint.
    nc = bacc.Bacc("TRN2", target_bir_lowering=False, debug=False, num_devices=8)

    qT = nc.dram_tensor("qT", [D, SQ], MM_DT, kind="ExternalInput")
    kT = nc.dram_tensor("kT", [D, S], MM_DT, kind="ExternalInput")
    vT = nc.dram_tensor("vT", [D, S], MM_DT, kind="ExternalInput")
    wqT = nc.dram_tensor("wqT", [D, D], MM_DT, kind="ExternalInput")
    wkT = nc.dram_tensor("wkT", [D, D], MM_DT, kind="ExternalInput")
    wvT = nc.dram_tensor("wvT", [D, D], MM_DT, kind="ExternalInput")
    bq = nc.dram_tensor("bq", [D], F32, kind="ExternalInput")
    bk = nc.dram_tensor("bk", [D], F32, kind="ExternalInput")
    bv = nc.dram_tensor("bv", [D], F32, kind="ExternalInput")
    maskS = nc.dram_tensor("maskS", [SQ, S], F32, kind="ExternalInput")
    out = nc.dram_tensor("out", [SQ, D], F32, kind="ExternalOutput")

    # [d, s] views with the 1024-wide d axis split into 8 partition blocks
    qT3 = qT.rearrange("(o p) s -> p o s", p=P)
    kT3 = kT.rearrange("(o p) s -> p o s", p=P)
    vT3 = vT.rearrange("(o p) s -> p o s", p=P)
    wqT3 = wqT.rearrange("(o p) e -> p o e", p=P)
    wkT3 = wkT.rearrange("(o p) e -> p o e", p=P)
    wvT3 = wvT.rearrange("(o p) e -> p o e", p=P)
    bq2 = bq.rearrange("(o p) -> p o", p=P)
    bk2 = bk.rearrange("(o p) -> p o", p=P)

    with tile.TileContext(nc) as tc:
        with (
            tc.tile_pool(name="const", bufs=1) as const_pool,
            tc.tile_pool(name="qk_res", bufs=1) as qk_res,
        ):
            identity = const_pool.tile([P, P], F32)
            make_identity(nc, identity)

            # Resident across all phases: keyT (64KB/part) + queryT (32KB/part)
            keyT_sb = qk_res.tile([P, NE, S], MM_DT, tag="keyT")
            queryT_sb = qk_res.tile([P, NE, SQ], MM_DT, tag="queryT")

            # ---- Phases 1+2: K and Q projections (transposed outputs)
            ps_proj_cm = tc.tile_pool(name="ps_proj", bufs=2, space="PSUM")
            ps_proj = ps_proj_cm.__enter__()
            with (
                tc.tile_pool(name="weights", bufs=1) as w_pool,
                tc.tile_pool(name="ins", bufs=2) as in_pool,
                tc.tile_pool(name="bias_qk", bufs=1) as bias_qk,
            ):
                bq_sb = bias_qk.tile([P, NE], F32, tag="bq")
                nc.gpsimd.dma_start(out=bq_sb, in_=bq2)
                bk_sb = bias_qk.tile([P, NE], F32, tag="bk")
                nc.gpsimd.dma_start(out=bk_sb, in_=bk2)

                for which in ("k", "q"):
                    w3, x3, b_sb, dst, n_sc = {
                        "k": (wkT3, kT3, bk_sb, keyT_sb, NKC),
                        "q": (wqT3, qT3, bq_sb, queryT_sb, SQ // KC),
                    }[which]
                    w_sb = w_pool.tile([P, NE, D], MM_DT, tag="w")
                    nc.sync.dma_start(out=w_sb, in_=w3)
                    for sc in range(n_sc):
                        xs = in_pool.tile([P, NE, KC], MM_DT, tag="ins")
                        nc.sync.dma_start(
                            out=xs, in_=x3[:, :, sc * KC:(sc + 1) * KC]
                        )
                        for eb in range(NE):
                            ps = ps_proj.tile([P, KC], F32)
                            for db in range(NE):
                                nc.tensor.matmul(
                                    ps,
                                    mm(w_sb[:, db, eb * P:(eb + 1) * P]),
                                    mm(xs[:, db, :]),
                                    start=(db == 0),
                                    stop=(db == NE - 1),
                                )
                            nc.scalar.activation(
                                out=dst[:, eb, sc * KC:(sc + 1) * KC],
                                in_=ps,
                                func=mybir.ActivationFunctionType.Identity,
                                bias=b_sb[:, eb:eb + 1],
                            )

            with tc.tile_pool(name="v_res", bufs=1) as v_res:
                value_sb = v_res.tile([P, NKB, D], MM_DT, tag="value")

                # ---- Phase 3: V projection -> value_sb [k, e] (natural)
                with (
                    tc.tile_pool(name="wv", bufs=1) as wv_pool,
                    tc.tile_pool(name="vins", bufs=2) as vin_pool,
                    tc.tile_pool(name="bias_v", bufs=1) as bias_v,
                ):
                    bv_bcast = bias_v.tile([P, D], F32, tag="bv")
                    nc.gpsimd.dma_start(
                        out=bv_bcast, in_=bv[None, :].to_broadcast([P, D])
                    )
                    for ec in range(2):
                        wv_sb = wv_pool.tile([P, NE, KC], MM_DT, tag="wv")
                        nc.sync.dma_start(
                            out=wv_sb, in_=wvT3[:, :, ec * KC:(ec + 1) * KC]
                        )
                        for kb in range(NKB):
                            vs = vin_pool.tile([P, NE, P], MM_DT, tag="vins")
                            nc.sync.dma_start(
                                out=vs, in_=vT3[:, :, kb * P:(kb + 1) * P]
                            )
                            ps = ps_proj.tile([P, KC], F32)
                            for db in range(NE):
                                nc.tensor.matmul(
                                    ps,
                                    mm(vs[:, db, :]),
                                    mm(wv_sb[:, db, :]),
                                    start=(db == 0),
                                    stop=(db == NE - 1),
                                )
                            nc.vector.tensor_add(
                                out=value_sb[:, kb, ec * KC:(ec + 1) * KC],
                                in0=ps,
                                in1=bv_bcast[:, ec * KC:(ec + 1) * KC],
                            )

                ps_proj_cm.__exit__(None, None, None)

                # ---- Phase 4: attention per q tile
                with (
                    tc.tile_pool(name="z", bufs=1) as z_pool,
                    tc.tile_pool(name="p", bufs=1) as p_pool,
                    tc.tile_pool(name="mask", bufs=2) as mask_pool,
                    tc.tile_pool(name="pt", bufs=3) as pt_pool,
                    tc.tile_pool(name="stats", bufs=4) as stat_pool,
                    tc.tile_pool(name="outs", bufs=2) as out_pool,
                    tc.tile_pool(name="ps_att", bufs=2, space="PSUM") as ps_att,
                    tc.tile_pool(name="ps_tr", bufs=2, space="PSUM") as ps_tr,
                    tc.tile_pool(name="ps_out", bufs=2, space="PSUM") as ps_out,
                ):
                    for j in range(NQT):
                        z_sb = z_pool.tile([P, S], F32, tag="z")
                        for kc in range(NKC):
                            ps_a = ps_att.tile([P, KC], F32)
                            for eb in range(NE):
                                nc.tensor.matmul(
                                    ps_a,
                                    mm(queryT_sb[:, eb, j * P:(j + 1) * P]),
                                    mm(keyT_sb[:, eb, kc * KC:(kc + 1) * KC]),
                                    start=(eb == 0),
                                    stop=(eb == NE - 1),
                                )
                            mask_t = mask_pool.tile([P, KC], F32, tag="mask")
                            nc.sync.dma_start(
                                out=mask_t,
                                in_=maskS[j * P:(j + 1) * P, kc * KC:(kc + 1) * KC],
                            )
                            # z = raw QK^T + mask*sqrt(D); evict PSUM -> SBUF
                            nc.vector.tensor_add(
                                out=z_sb[:, kc * KC:(kc + 1) * KC],
                                in0=ps_a,
                                in1=mask_t,
                            )

                        m_t = stat_pool.tile([P, 1], F32, tag="m")
                        nc.vector.reduce_max(m_t, z_sb, axis=mybir.AxisListType.X)
                        negm = stat_pool.tile([P, 1], F32, tag="negm")
                        nc.scalar.mul(out=negm, in_=m_t, mul=-SCALE)
                        l_t = stat_pool.tile([P, 1], F32, tag="l")
                        p_sb = p_pool.tile([P, S], F32, tag="p")
                        # p = exp(z/sqrt(D) - m/sqrt(D)); l = rowsum(p)
                        nc.scalar.activation(
                            out=p_sb,
                            in_=z_sb,
                            func=mybir.ActivationFunctionType.Exp,
                            bias=negm,
                            scale=SCALE,
                            accum_out=l_t,
                        )
                        recip_l = stat_pool.tile([P, 1], F32, tag="recip")
                        nc.vector.reciprocal(recip_l, l_t)

                        ps_o = ps_out.tile([P, D], F32)
                        for kb in range(NKB):
                            ps_t = ps_tr.tile([P, P], F32)
                            nc.tensor.transpose(
                                ps_t, p_sb[:, kb * P:(kb + 1) * P], identity
                            )
                            pT_sb = pt_pool.tile([P, P], MM_DT, tag="pt")
                            nc.vector.tensor_copy(pT_sb, ps_t)
                            for ec in range(2):
                                nc.tensor.matmul(
                                    ps_o[:, ec * KC:(ec + 1) * KC],
                                    mm(pT_sb),
                                    mm(value_sb[:, kb, ec * KC:(ec + 1) * KC]),
                                    start=(kb == 0),
                                    stop=(kb == NKB - 1),
                                )
                        out_sb = out_pool.tile([P, D], F32, tag="out")
                        nc.vector.tensor_scalar_mul(
                            out=out_sb, in0=ps_o, scalar1=recip_l
                        )
                        nc.sync.dma_start(out=out[j * P:(j + 1) * P, :], in_=out_sb)

    nc.finalize()
    return nc


_NC_CACHE = None
LAST_RESULT = None  # BassKernelResults from the most recent kernel() call


def kernel(q, k, v, mask, w_q, b_q, w_k, b_k, w_v, b_v):
    global _NC_CACHE, LAST_RESULT
    if _NC_CACHE is None:
        _NC_CACHE = build_bass()
    nc = _NC_CACHE

    f32 = np.float32
    wqT = np.ascontiguousarray(np.asarray(w_q, dtype=f32).T)
    wkT = np.ascontiguousarray(np.asarray(w_k, dtype=f32).T)
    wvT = np.ascontiguousarray(np.asarray(w_v, dtype=f32).T)
    bq = np.ascontiguousarray(np.asarray(b_q, dtype=f32))
    bk = np.ascontiguousarray(np.asarray(b_k, dtype=f32))
    bv = np.ascontiguousarray(np.asarray(b_v, dtype=f32))
    mask = np.asarray(mask, dtype=f32)
    # pre-scale so the kernel can add it to raw QK^T before the fused 1/sqrt(D)
    maskS_halves = [
        np.ascontiguousarray(mask[0, h * SQ:(h + 1) * SQ, :] * f32(math.sqrt(D)))
        for h in range(2)
    ]

    in_maps = []
    for c in range(8):
        b, h = c // 2, c % 2
        rows = slice(h * SQ, (h + 1) * SQ)
        in_maps.append({
            "qT": np.ascontiguousarray(np.asarray(q[b], dtype=f32)[rows, :].T),
            "kT": np.ascontiguousarray(np.asarray(k[b], dtype=f32).T),
            "vT": np.ascontiguousarray(np.asarray(v[b], dtype=f32).T),
            "wqT": wqT, "wkT": wkT, "wvT": wvT,
            "bq": bq, "bk": bk, "bv": bv,
            "maskS": maskS_halves[h],
        })

    res = run_bass_kernel_spmd(nc, in_maps, list(range(8)))
    LAST_RESULT = res

    out = np.empty((B, S, D), dtype=f32)
    for c in range(8):
        b, h = c // 2, c % 2
        out[b, h * SQ:(h + 1) * SQ, :] = res.results[c]["out"]
    return out


# revision 19
# speedup vs baseline: 2.6463x; 1.0506x over previous
"""Trainium2 Bass kernel for single-head attention with QKV projections.

Problem: q,k,v [4, 2048, 1024] fp32; w_q/w_k/w_v [1024, 1024]; b_* [1024];
additive mask [1, 2048, 2048].
  query = q @ w_q.T + b_q ; key = k @ w_k.T + b_k ; value = v @ w_v.T + b_v
  att = softmax(query @ key.T / sqrt(D) + mask) ; out = att @ value

Sharding: 8 cores = 4 batches x 2 sequence-halves of q rows (1024 rows per
core). Each core computes the full K/V projection for its batch (duplicated
across the pair) and the attention output for its q rows. Uniform SPMD
program; the mask is applied as data (no causality assumption).

Host-side prep (free, not on-device): transposes q/k/v and the weights so
the contraction dim lands on SBUF partitions, and pre-scales the mask by
sqrt(D) so it can be added to the raw QK^T product before the 1/sqrt(D)
scaling fused into the exp activation.
"""

import math

import numpy as np

import concourse.bass as bass
import concourse.mybir as mybir
import concourse.tile as tile
from concourse import bacc
from concourse.bass_utils import run_bass_kernel_spmd
from concourse.masks import make_identity

B, S, D = 4, 2048, 1024
SQ = S // 2          # q rows per core
P = 128              # partitions
NE = D // P          # 8 feature blocks
NQT = SQ // P        # 8 q tiles per core
NKB = S // P         # 16 key blocks
KC = 512             # key chunk for QK^T matmuls
NKC = S // KC        # 4 key chunks
SCALE = 1.0 / math.sqrt(D)

F32 = mybir.dt.float32
# Matmul operand dtype: float32 = full precision (4 cyc/row);
# float32r = single-pass reduced-precision multiply (1 cyc/row at N>=256).
MM_DT = mybir.dt.float32r


def mm(ap):
    """Matmul operands are typed MM_DT end-to-end; no view change needed."""
    return ap


def build_bass():
    # Bacc (not raw Bass): its compile() pass legalizes semaphore waits
    # (move_matmul_waits_to_ldweights + generate_event_semaphores) for the
    # TRN2 1-wait-per-instruction constraint.
    nc = bacc.Bacc("TRN2", target_bir_lowering=False, debug=False, num_devices=8)

    qT = nc.dram_tensor("qT", [D, SQ], MM_DT, kind="ExternalInput")
    kT = nc.dram_tensor("kT", [D, S], MM_DT, kind="ExternalInput")
    vT = nc.dram_tensor("vT", [D, S], MM_DT, kind="ExternalInput")
    wqT = nc.dram_tensor("wqT", [D, D], MM_DT, kind="ExternalInput")
    wkT = nc.dram_tensor("wkT", [D, D], MM_DT, kind="ExternalInput")
    wvT = nc.dram_tensor("wvT", [D, D], MM_DT, kind="ExternalInput")
    bq = nc.dram_tensor("bq", [D], F32, kind="ExternalInput")
    bk = nc.dram_tensor("bk", [D], F32, kind="ExternalInput")
    bv = nc.dram_tensor("bv", [D], F32, kind="ExternalInput")
    maskS = nc.dram_tensor("maskS", [SQ, S], F32, kind="ExternalInput")
    out = nc.dram_tensor("out", [SQ, D], F32, kind="ExternalOutput")

    # [d, s] views with the 1024-wide d axis split into 8 partition blocks
    qT3 = qT.rearrange("(o p) s -> p o s", p=P)
    kT3 = kT.rearrange("(o p) s -> p o s", p=P)
    vT3 = vT.rearrange("(o p) s -> p o s", p=P)
    wqT3 = wqT.rearrange("(o p) e -> p o e", p=P)
    wkT3 = wkT.rearrange("(o p) e -> p o e", p=P)
    wvT3 = wvT.rearrange("(o p) e -> p o e", p=P)
    bq2 = bq.rearrange("(o p) -> p o", p=P)
    bk2 = bk.rearrange("(o p) -> p o", p=P)

    with tile.TileContext(nc) as tc:
        with (
            tc.tile_pool(name="const", bufs=1) as const_pool,
            tc.tile_pool(name="qk_res", bufs=1) as qk_res,
        ):
            identity = const_pool.tile([P, P], F32)
            make_identity(nc, identity)

            # Resident: queryT (32KB/part) + keyT (64KB/part)
            queryT_sb = qk_res.tile([P, NE, SQ], MM_DT, tag="queryT")
            keyT_sb = qk_res.tile([P, NE, S], MM_DT, tag="keyT")

            # ---- Phases 1+2: Q then K projections (transposed outputs).
            # Weights stream as [D, 128] e-panels; all s-chunks of the input
            # stay resident so each (w-panel, d-block) stationary operand
            # feeds n_sc back-to-back matmuls (weight-reload amortized).
            ps_proj_cm = tc.tile_pool(name="ps_proj", bufs=8, space="PSUM")
            ps_proj = ps_proj_cm.__enter__()
            with (
                tc.tile_pool(name="wpan", bufs=2) as wpan_pool,
                tc.tile_pool(name="ins", bufs=4) as in_pool,
                tc.tile_pool(name="bias_qk", bufs=1) as bias_qk,
            ):
                bq_sb = bias_qk.tile([P, NE], F32, tag="bq")
                nc.gpsimd.dma_start(out=bq_sb, in_=bq2)
                bk_sb = bias_qk.tile([P, NE], F32, tag="bk")
                nc.gpsimd.dma_start(out=bk_sb, in_=bk2)

                for which in ("q", "k"):
                    w3, x3, b_sb, dst, n_sc = {
                        "k": (wkT3, kT3, bk_sb, keyT_sb, NKC),
                        "q": (wqT3, qT3, bq_sb, queryT_sb, SQ // KC),
                    }[which]
                    xs = []
                    for sc in range(n_sc):
                        x_t = in_pool.tile([P, NE, KC], MM_DT, tag="ins")
                        nc.sync.dma_start(
                            out=x_t, in_=x3[:, :, sc * KC:(sc + 1) * KC]
                        )
                        xs.append(x_t)
                    for eb in range(NE):
                        w_t = wpan_pool.tile([P, NE, P], MM_DT, tag="wpan")
                        nc.sync.dma_start(out=w_t, in_=w3[:, :, eb * P:(eb + 1) * P])
                        pss = [ps_proj.tile([P, KC], F32, name="ps", tag="ps_proj") for _ in range(n_sc)]
                        for db in range(NE):
                            for sc in range(n_sc):
                                nc.tensor.matmul(
                                    pss[sc],
                                    w_t[:, db, :],
                                    xs[sc][:, db, :],
                                    start=(db == 0),
                                    stop=(db == NE - 1),
                                )
                        for sc in range(n_sc):
                            nc.scalar.activation(
                                out=dst[:, eb, sc * KC:(sc + 1) * KC],
                                in_=pss[sc],
                                func=mybir.ActivationFunctionType.Identity,
                                bias=b_sb[:, eb:eb + 1],
                            )

            with tc.tile_pool(name="v_res", bufs=1) as v_res:
                value_sb = v_res.tile([P, NKB, D], MM_DT, tag="value")

                # ---- Phase 3: V projection -> value_sb [k, e] (natural)
                with (
                    tc.tile_pool(name="wv", bufs=1) as wv_pool,
                    tc.tile_pool(name="vins", bufs=2) as vin_pool,
                    tc.tile_pool(name="bias_v", bufs=1) as bias_v,
                ):
                    bv_bcast = bias_v.tile([P, D], F32, tag="bv")
                    nc.gpsimd.dma_start(
                        out=bv_bcast, in_=bv[None, :].to_broadcast([P, D])
                    )
                    for ec in range(2):
                        wv_sb = wv_pool.tile([P, NE, KC], MM_DT, tag="wv")
                        nc.sync.dma_start(
                            out=wv_sb, in_=wvT3[:, :, ec * KC:(ec + 1) * KC]
                        )
                        for kb in range(NKB):
                            vs = vin_pool.tile([P, NE, P], MM_DT, tag="vins")
                            nc.sync.dma_start(
                                out=vs, in_=vT3[:, :, kb * P:(kb + 1) * P]
                            )
                            ps = ps_proj.tile([P, KC], F32, tag="ps_proj")
                            for db in range(NE):
                                nc.tensor.matmul(
                                    ps,
                                    vs[:, db, :],
                                    wv_sb[:, db, :],
                                    start=(db == 0),
                                    stop=(db == NE - 1),
                                )
                            nc.vector.tensor_add(
                                out=value_sb[:, kb, ec * KC:(ec + 1) * KC],
                                in0=ps,
                                in1=bv_bcast[:, ec * KC:(ec + 1) * KC],
                            )

                ps_proj_cm.__exit__(None, None, None)

                # ---- Phase 4: attention per q tile
                with (
                    tc.tile_pool(name="z", bufs=1) as z_pool,
                    tc.tile_pool(name="p", bufs=1) as p_pool,
                    tc.tile_pool(name="mask", bufs=4) as mask_pool,
                    tc.tile_pool(name="pt", bufs=3) as pt_pool,
                    tc.tile_pool(name="stats", bufs=4) as stat_pool,
                    tc.tile_pool(name="outs", bufs=1) as out_pool,
                    tc.tile_pool(name="ps_att", bufs=4, space="PSUM") as ps_att,
                    tc.tile_pool(name="ps_tr", bufs=2, space="PSUM") as ps_tr,
                    tc.tile_pool(name="ps_out", bufs=1, space="PSUM") as ps_out,
                ):
                    for j in range(NQT):
                        z_sb = z_pool.tile([P, S], F32, tag="z")
                        # QK^T: one queryT stationary block feeds all 4 key
                        # chunks back-to-back (4 PSUM banks accumulate).
                        pss_a = [ps_att.tile([P, KC], F32, name="ps_a", tag="ps_att") for _ in range(NKC)]
                        for eb in range(NE):
                            for kc in range(NKC):
                                nc.tensor.matmul(
                                    pss_a[kc],
                                    queryT_sb[:, eb, j * P:(j + 1) * P],
                                    keyT_sb[:, eb, kc * KC:(kc + 1) * KC],
                                    start=(eb == 0),
                                    stop=(eb == NE - 1),
                                )
                        m4 = stat_pool.tile([P, NKC], F32, tag="m4")
                        for kc in range(NKC):
                            mask_t = mask_pool.tile([P, KC], F32, tag="mask")
                            nc.sync.dma_start(
                                out=mask_t,
                                in_=maskS[j * P:(j + 1) * P, kc * KC:(kc + 1) * KC],
                            )
                            # z = raw QK^T + mask*sqrt(D)
                            nc.vector.tensor_add(
                                out=z_sb[:, kc * KC:(kc + 1) * KC],
                                in0=pss_a[kc],
                                in1=mask_t,
                            )
                            nc.vector.reduce_max(
                                m4[:, kc:kc + 1],
                                z_sb[:, kc * KC:(kc + 1) * KC],
                                axis=mybir.AxisListType.X,
                            )

                        m_t = stat_pool.tile([P, 1], F32, tag="m")
                        nc.vector.reduce_max(m_t, m4, axis=mybir.AxisListType.X)
                        negm = stat_pool.tile([P, 1], F32, tag="negm")
                        nc.vector.tensor_scalar_mul(
                            out=negm, in0=m_t, scalar1=-SCALE
                        )
                        l_t = stat_pool.tile([P, 1], F32, tag="l")
                        p_sb = p_pool.tile([P, S], F32, tag="p")
                        # p = exp(z/sqrt(D) - m/sqrt(D)); l = rowsum(p)
                        nc.scalar.activation(
                            out=p_sb,
                            in_=z_sb,
                            func=mybir.ActivationFunctionType.Exp,
                            bias=negm,
                            scale=SCALE,
                            accum_out=l_t,
                        )
                        recip_l = stat_pool.tile([P, 1], F32, tag="recip")
                        nc.vector.reciprocal(recip_l, l_t)

                        ps_o = ps_out.tile([P, D], F32, tag="ps_out")
                        for kb in range(NKB):
                            ps_t = ps_tr.tile([P, P], F32, tag="ps_tr")
                            nc.tensor.transpose(
                                ps_t, p_sb[:, kb * P:(kb + 1) * P], identity
                            )
                            pT_sb = pt_pool.tile([P, P], MM_DT, tag="pt")
                            nc.vector.tensor_copy(pT_sb, ps_t)
                            for ec in range(2):
                                nc.tensor.matmul(
                                    ps_o[:, ec * KC:(ec + 1) * KC],
                                    pT_sb,
                                    value_sb[:, kb, ec * KC:(ec + 1) * KC],
                                    start=(kb == 0),
                                    stop=(kb == NKB - 1),
                                )
                        out_sb = out_pool.tile([P, D], F32, tag="out")
                        nc.vector.tensor_scalar_mul(
                            out=out_sb, in0=ps_o, scalar1=recip_l
                        )
                        nc.sync.dma_start(out=out[j * P:(j + 1) * P, :], in_=out_sb)

    nc.finalize()
    return nc


_NC_CACHE = None
LAST_RESULT = None  # BassKernelResults from the most recent kernel() call


def kernel(q, k, v, mask, w_q, b_q, w_k, b_k, w_v, b_v):
    global _NC_CACHE, LAST_RESULT
    if _NC_CACHE is None:
        _NC_CACHE = build_bass()
    nc = _NC_CACHE

    f32 = np.float32
    wqT = np.ascontiguousarray(np.asarray(w_q, dtype=f32).T)
    wkT = np.ascontiguousarray(np.asarray(w_k, dtype=f32).T)
    wvT = np.ascontiguousarray(np.asarray(w_v, dtype=f32).T)
    bq = np.ascontiguousarray(np.asarray(b_q, dtype=f32))
    bk = np.ascontiguousarray(np.asarray(b_k, dtype=f32))
    bv = np.ascontiguousarray(np.asarray(b_v, dtype=f32))
    mask = np.asarray(mask, dtype=f32)
    # pre-scale so the kernel can add it to raw QK^T before the fused 1/sqrt(D)
    maskS_halves = [
        np.ascontiguousarray(mask[0, h * SQ:(h + 1) * SQ, :] * f32(math.sqrt(D)))
        for h in range(2)
    ]

    in_maps = []
    for c in range(8):
        b, h = c // 2, c % 2
        rows = slice(h * SQ, (h + 1) * SQ)
        in_maps.append({
            "qT": np.ascontiguousarray(np.asarray(q[b], dtype=f32)[rows, :].T),
            "kT": np.ascontiguousarray(np.asarray(k[b], dtype=f32).T),
            "vT": np.ascontiguousarray(np.asarray(v[b], dtype=f32).T),
            "wqT": wqT, "wkT": wkT, "wvT": wvT,
            "bq": bq, "bk": bk, "bv": bv,
            "maskS": maskS_halves[h],
        })

    res = run_bass_kernel_spmd(nc, in_maps, list(range(8)))
    LAST_RESULT = res

    out = np.empty((B, S, D), dtype=f32)
    for c in range(8):
        b, h = c // 2, c % 2
        out[b, h * SQ:(h + 1) * SQ, :] = res.results[c]["out"]
    return out
